# revision 19
# baseline (speedup 1.0000x reference)
"""Trainium2 Bass kernel for causal GQA multi-head attention (nn_MHA_79362405695575).

Full (unsharded) inputs -> full output. Internally: tensor-parallel over heads
across 8 NeuronCores for projections+attention. Core c owns q-heads [4c,4c+4)
and kv-head c. After attention, a single AllToAll redistributes the attention
outputs from head-sharded to row-sharded (4.2 MB instead of ReduceScattering
33.5 MB of out-proj partials); each core then computes its 512 output rows
against the full (streamed) Wo with no further collective.

Reference semantics (fp32):
  q = x@Wq; k = x@Wk; v = x@Wv + bv           (B=2, S=2048, D=2048)
  q,k := interleaved RoPE(base 10000, hd=64)
  scores = q k^T / 8 (causal), attn = softmax
  out = attn @ v;  y = out @ Wo + bo

All matmuls run as float32r (TF32-class, ~2e-4 rel err, full PE rate).
Everything on-chip is transposed: qT/kT/vT [dim, row] layouts so no PE
transposes are needed anywhere in attention. Softmax is max-free (scores are
provably small) and denominators ride along the AV matmul as a 65th column
of v. Inputs arrive pre-tiled from the host so every DMA is a few large
contiguous transfers.
"""

import numpy as np

import concourse.bass as bass
import concourse.tile as tile
from concourse import bacc, mybir
from concourse.bass_utils import run_bass_kernel_spmd

# ---- problem constants (hardcoded; kernel.py must be self-contained) ----
B, S, D = 2, 2048, 2048
NH, NKV, HD = 32, 8, 64
ROPE_BASE = 10000.0
NC = 8                    # cores
HPC = NH // NC            # q heads per core = 4
R = B * S                 # 4096 rows
RS_N = 8                  # projection row spans
RS_W = R // RS_N          # 512 rows per span
QS_W = 512                # attention q-span width
QS_N = S // QS_W          # 4 q spans per batch
KB_W = 128                # k block width
NKB = S // KB_W           # 16 k blocks per batch
OB = D // 128             # 16 out-proj column blocks

F32 = mybir.dt.float32
F32R = mybir.dt.float32r

_CACHE = {}


def _build():
    nc = bacc.Bacc("TRN2", target_bir_lowering=False, debug=False, num_devices=NC)

    # ---- DRAM I/O (pre-tiled on host) ----
    xta = nc.dram_tensor("xta", [RS_N, 128, 8, RS_W], F32R, kind="ExternalInput").ap()
    xtb = nc.dram_tensor("xtb", [RS_N, 128, 8, RS_W], F32R, kind="ExternalInput").ap()
    wq = nc.dram_tensor("wq", [128, D // 128, 256], F32R, kind="ExternalInput").ap()
    wkv = nc.dram_tensor("wkv", [128, D // 128, 128], F32R, kind="ExternalInput").ap()
    wo = nc.dram_tensor("wo", [128, OB, OB, 128], F32R, kind="ExternalInput").ap()
    bv_in = nc.dram_tensor("bv", [HD, 1], F32, kind="ExternalInput").ap()
    bo_in = nc.dram_tensor("bo", [128, OB], F32, kind="ExternalInput").ap()
    c4h = nc.dram_tensor("c4h", [128, S], F32, kind="ExternalInput").ap()
    s4h = nc.dram_tensor("s4h", [128, S], F32, kind="ExternalInput").ap()
    p2 = nc.dram_tensor("p2", [128, 128], F32R, kind="ExternalInput").ap()
    ident = nc.dram_tensor("ident", [64, 64], F32R, kind="ExternalInput").ap()
    masks = nc.dram_tensor("masks", [128, 4, HPC * QS_W], F32R, kind="ExternalInput").ap()
    ones32 = nc.dram_tensor("ones32", [128, (R // KB_W) * 64], F32R,
                            kind="ExternalInput").ap()
    y_sh = nc.dram_tensor("y_sh", [D, RS_W], F32, kind="ExternalOutput").ap()

    DMA = nc.sync

    with tile.TileContext(nc) as tc:
        with (
            tc.tile_pool(name="persist", bufs=1) as pp,
            tc.tile_pool(name="dram", bufs=1, space="DRAM") as dram,
        ):
            # ---- persistent SBUF (whole kernel) ----
            qrT = [pp.tile([128, R], F32R, tag=f"qrT{t}", name=f"qrT{t}") for t in range(2)]
            krT = pp.tile([128, R], F32R, tag="krT")
            # v_aug cols 0:64 = ones (so AV materializes the softmax
            # denominator on partitions 0:64 of pav), cols 64:128 = v.
            v_aug = pp.tile([128, R // KB_W, 128], F32R, tag="vaug")
            p2_sb = pp.tile([128, 128], F32R, tag="p2")
            id_sb = pp.tile([64, 64], F32R, tag="ident")
            bv_sb = pp.tile([HD, 1], F32, tag="bv")
            bo_sb = pp.tile([128, OB], F32, tag="bo")

            DMA.dma_start(out=p2_sb[:], in_=p2[:])
            DMA.dma_start(out=id_sb[:], in_=ident[:])
            DMA.dma_start(out=bv_sb[:], in_=bv_in[:])
            DMA.dma_start(out=bo_sb[:], in_=bo_in[:])
            DMA.dma_start(out=v_aug[:, :, 0:64],
                          in_=ones32.rearrange("p (j o) -> p j o", o=64))

            # AllToAll buffers: shard j of a2a_in = this core's 2 head-pair
            # tiles for row-block j; after A2A, a2a_out[j, g] = core j's
            # head-pair g for THIS core's row-block.
            a2a_in = dram.tile([NC, 2, 128, RS_W], F32R)
            a2a_out = dram.tile([NC, 2, 128, RS_W], F32R)

            # ================= stage 1: projections + RoPE =================
            with (
                tc.tile_pool(name="w1p", bufs=1) as w1p,
                tc.tile_pool(name="xtpa", bufs=2) as xtpa,
                tc.tile_pool(name="xtpb", bufs=2) as xtpb,
                tc.tile_pool(name="ropet", bufs=2) as ropet,
                tc.tile_pool(name="vstg", bufs=2) as vstg,
                tc.tile_pool(name="ps_q", bufs=2, space="PSUM") as ps_q,
                tc.tile_pool(name="ps_kv", bufs=2, space="PSUM") as ps_kv,
                tc.tile_pool(name="ps_sw", bufs=2, space="PSUM") as ps_sw,
                tc.tile_pool(name="ps_vt", bufs=2, space="PSUM") as ps_vt,
            ):
                wq_sb = w1p.tile([128, D // 128, 256], F32R, tag="wq")
                wkv_sb = w1p.tile([128, D // 128, 128], F32R, tag="wkv")
                c4_sb = w1p.tile([128, S], F32, tag="c4")
                s4_sb = w1p.tile([128, S], F32, tag="s4")
                DMA.dma_start(out=wq_sb[:], in_=wq[:])
                DMA.dma_start(out=wkv_sb[:], in_=wkv[:])
                DMA.dma_start(out=c4_sb[:], in_=c4h[:])
                DMA.dma_start(out=s4_sb[:], in_=s4h[:])
                SPB = RS_N // B          # spans per batch
                for rs in range(RS_N):
                    rsl = slice(rs * RS_W, (rs + 1) * RS_W)
                    ssl = slice((rs % SPB) * RS_W, (rs % SPB + 1) * RS_W)
                    xa = xtpa.tile([128, 8, RS_W], F32R, tag="xa")
                    xb = xtpb.tile([128, 8, RS_W], F32R, tag="xb")
                    DMA.dma_start(out=xa[:], in_=xta[rs])
                    DMA.dma_start(out=xb[:], in_=xtb[rs])

                    def xt(kb):
                        return xa[:, kb, :] if kb < 8 else xb[:, kb - 8, :]

                    # -- q projection: 2 colblocks (2 heads each) --
                    for cb in range(2):
                        pq = ps_q.tile([128, RS_W], F32, tag="pq")
                        for kb in range(D // 128):
                            nc.tensor.matmul(pq[:], wq_sb[:, kb, cb * 128:(cb + 1) * 128],
                                             xt(kb),
                                             start=(kb == 0), stop=(kb == D // 128 - 1))
                        # RoPE: qr = pq*C + P2.T @ (pq*S)
                        st = ropet.tile([128, RS_W], F32R, tag="st")
                        nc.vector.tensor_tensor(out=st[:], in0=pq[:], in1=s4_sb[:, ssl],
                                                op=mybir.AluOpType.mult)
                        sw = ps_sw.tile([128, RS_W], F32, tag="sw")
                        nc.tensor.matmul(sw[:], p2_sb[:], st[:], start=True, stop=True)
                        ct = ropet.tile([128, RS_W], F32, tag="ct")
                        nc.vector.tensor_tensor(out=ct[:], in0=pq[:], in1=c4_sb[:, ssl],
                                                op=mybir.AluOpType.mult)
                        nc.vector.tensor_tensor(out=qrT[cb][:, rsl], in0=ct[:], in1=sw[:],
                                                op=mybir.AluOpType.add)

                    # -- kv projection: cols 0:64 = kT(perm), 64:128 = vT --
                    pkv = ps_kv.tile([128, RS_W], F32, tag="pkv")
                    for kb in range(D // 128):
                        nc.tensor.matmul(pkv[:], wkv_sb[:, kb, :], xt(kb),
                                         start=(kb == 0), stop=(kb == D // 128 - 1))
                    # k RoPE (partitions 0:64), duplicated into krT[0:64] and [64:128]
                    stk = ropet.tile([64, RS_W], F32R, tag="stk")
                    nc.vector.tensor_tensor(out=stk[:], in0=pkv[0:64, :],
                                            in1=s4_sb[0:64, ssl], op=mybir.AluOpType.mult)
                    swk = ps_sw.tile([64, RS_W], F32, tag="sw")
                    nc.tensor.matmul(swk[:], p2_sb[0:64, 0:64], stk[:], start=True, stop=True)
                    ctk = ropet.tile([64, RS_W], F32, tag="ctk")
                    nc.vector.tensor_tensor(out=ctk[:], in0=pkv[0:64, :],
                                            in1=c4_sb[0:64, ssl], op=mybir.AluOpType.mult)
                    nc.vector.tensor_tensor(out=krT[0:64, rsl], in0=ctk[:], in1=swk[:],
                                            op=mybir.AluOpType.add)
                    nc.vector.tensor_tensor(out=krT[64:128, rsl], in0=ctk[:], in1=swk[:],
                                            op=mybir.AluOpType.add)

                    # v: bias add then transpose [64,128] -> [128,64] blocks
                    vst = vstg.tile([64, RS_W], F32R, tag="vst")
                    nc.scalar.activation(out=vst[:], in_=pkv[64:128, :],
                                         func=mybir.ActivationFunctionType.Identity,
                                         bias=bv_sb[:], scale=1.0)
                    for j in range(RS_W // KB_W):
                        pv = ps_vt.tile([128, 64], F32R, tag="pv")
                        nc.tensor.transpose(pv[:], vst[:, j * 128:(j + 1) * 128], id_sb[:])
                        nc.vector.tensor_copy(
                            out=v_aug[:, rs * (RS_W // KB_W) + j, 64:128], in_=pv[:])

            # ============ stage 2: attention -> per-span A2A shards ============
            with tc.tile_pool(name="w2p", bufs=1) as w2p:
                mask_sb = w2p.tile([128, 4, HPC * QS_W], F32R, tag="masks")
                DMA.dma_start(out=mask_sb[:], in_=masks[:])

                with (
                    tc.tile_pool(name="ptp", bufs=4) as ptp,
                    tc.tile_pool(name="normp", bufs=2) as normp,
                    tc.tile_pool(name="otp", bufs=4) as otp,
                    tc.tile_pool(name="ps_s", bufs=2, space="PSUM") as ps_s,
                    tc.tile_pool(name="ps_av", bufs=2, space="PSUM") as ps_av,
                ):
                    for b in range(B):
                        for qs in range(QS_N):
                            jrow = SPB * b + qs
                            n_kb = 4 * (qs + 1)
                            qsl = slice(b * S + qs * QS_W, b * S + (qs + 1) * QS_W)
                            pav = [ps_av.tile([128, 2 * QS_W], F32, tag="pav",
                                              name=f"pav{b}{qs}{g}") for g in range(2)]
                            # software pipeline, scalar-paced: per (kb, g) the
                            # 2 score MMs write a double-buffered [128,1024]
                            # PSUM tile so scores(kb+1) overlap exp(kb); the
                            # AV MMs for (kb-1) are issued after scores(kb)
                            # and run under exp(kb).
                            pts = [[None] * 2 for _ in range(n_kb)]
                            for kb in range(n_kb + 1):
                                if kb < n_kb:
                                    kbl = slice(b * S + kb * KB_W,
                                                b * S + (kb + 1) * KB_W)
                                    dlt = kb - 4 * qs
                                    for g in range(2):
                                        pss = ps_s.tile([128, 2 * QS_W], F32,
                                                        tag="pss")
                                        for u in range(2):
                                            nc.tensor.matmul(
                                                pss[:, u * QS_W:(u + 1) * QS_W],
                                                krT[u * 64:(u + 1) * 64, kbl],
                                                qrT[g][u * 64:(u + 1) * 64, qsl],
                                                start=True, stop=True)
                                        pt = ptp.tile([128, 2 * QS_W], F32R,
                                                      tag="pt")
                                        nc.scalar.activation(
                                            out=pt[:], in_=pss[:],
                                            func=mybir.ActivationFunctionType.Exp,
                                            scale=float(HD) ** -0.5)
                                        if dlt >= 0:
                                            nc.vector.tensor_tensor(
                                                out=pt[:], in0=pt[:],
                                                in1=mask_sb[:, dlt, 0:2 * QS_W],
                                                op=mybir.AluOpType.mult)
                                        pts[kb][g] = pt
                                if kb > 0:
                                    for g in range(2):
                                        pt1 = pts[kb - 1][g]
                                        for u in range(2):
                                            nc.tensor.matmul(
                                                pav[g][:, u * QS_W:(u + 1) * QS_W],
                                                v_aug[:, b * NKB + kb - 1, :],
                                                pt1[:, u * QS_W:(u + 1) * QS_W],
                                                start=(kb == 1), stop=(kb == n_kb))
                            # normalize: pav rows 0:64 = denominator (dup x64),
                            # rows 64:128 = out values. All on vector.
                            for g in range(2):
                                # custom-DVE ops: HW-safe only from SBUF at
                                # partition 0 (probed) — stage den via SBUF
                                dsb = normp.tile([64, 2 * QS_W], F32, tag="dsb")
                                nc.vector.tensor_copy(out=dsb[:], in_=pav[g][0:64, :])
                                rbs = normp.tile([64, 2 * QS_W], F32, tag="rbs")
                                nc.vector.reciprocal_approx_fast(
                                    out=rbs[:], in_=dsb[:])
                                ot = otp.tile([128, QS_W], F32R, tag="ot")
                                for u in range(2):
                                    nc.vector.tensor_tensor(
                                        out=ot[u * 64:(u + 1) * 64, :],
                                        in0=pav[g][64:128, u * QS_W:(u + 1) * QS_W],
                                        in1=rbs[:, u * QS_W:(u + 1) * QS_W],
                                        op=mybir.AluOpType.mult)
                                DMA.dma_start(out=a2a_in[jrow, g], in_=ot[:])

            # ---- stage 2.5: redistribute heads -> rows ----
            nc.gpsimd.collective_compute(
                "AllToAll", mybir.AluOpType.bypass,
                replica_groups=[list(range(NC))],
                ins=[a2a_in[:]], outs=[a2a_out[:]],
            )

            # ---- stage 3: local out-projection over full Wo ----
            with (
                tc.tile_pool(name="w3p", bufs=1) as w3p,
                tc.tile_pool(name="wop", bufs=2) as wop,
                tc.tile_pool(name="ystg", bufs=4) as ystg,
                tc.tile_pool(name="ps_y", bufs=3, space="PSUM") as ps_y,
                tc.tile_pool(name="ps_kw", bufs=1, space="PSUM") as ps_kw,
            ):
                # keep the PE's HAM clock warm while the A2A runs: ~400
                # dependency-free matmuls on the tensor queue fill the
                # otherwise-idle collective window so stage 3 starts at
                # full clock instead of 1.2 GHz.
                kw = ps_kw.tile([128, 2 * KB_W], F32, tag="kw")
                for _ in range(650):
                    nc.tensor.matmul(kw[:], p2_sb[:],
                                     qrT[0][:, 0:2 * KB_W],
                                     start=True, stop=True)
                of_sb = w3p.tile([128, OB, RS_W], F32R, tag="of")
                for dd in range(OB):
                    DMA.dma_start(out=of_sb[:, dd, :], in_=a2a_out[dd // 2, dd % 2])
                for o in range(OB):
                    wo_sb = wop.tile([128, OB, 128], F32R, tag="wo")
                    DMA.dma_start(out=wo_sb[:], in_=wo[:, o])
                    py = ps_y.tile([128, RS_W], F32, tag="py")
                    for dd in range(OB):
                        nc.tensor.matmul(py[:], wo_sb[:, dd, :], of_sb[:, dd, :],
                                         start=(dd == 0), stop=(dd == OB - 1))
                    ys = ystg.tile([128, RS_W], F32, tag="ys")
                    nc.scalar.activation(out=ys[:], in_=py[:],
                                         func=mybir.ActivationFunctionType.Identity,
                                         bias=bo_sb[:, o:o + 1], scale=1.0)
                    DMA.dma_start(out=y_sh[o * 128:(o + 1) * 128, :], in_=ys[:])

    nc.finalize()
    return nc


def _rope_perm():
    return np.concatenate([np.arange(0, HD, 2), np.arange(1, HD, 2)])


def _host_prep(x, Wq, Wk, Wv, bv, Wo, bo):
    """Build per-core input maps (inputs pre-tiled to SBUF layouts)."""
    perm = _rope_perm()

    # x tiled: A[kb, p, r] = x[r, kb*128+p];  xta = kb 0..7, xtb = kb 8..15
    A = np.ascontiguousarray(x.reshape(R, D).T).reshape(D // 128, 128, R)
    xta = np.ascontiguousarray(
        A[0:8].reshape(8, 128, RS_N, RS_W).transpose(2, 1, 0, 3)).astype(np.float32)
    xtb = np.ascontiguousarray(
        A[8:16].reshape(8, 128, RS_N, RS_W).transpose(2, 1, 0, 3)).astype(np.float32)

    theta = (1.0 / ROPE_BASE ** (np.arange(0, HD, 2, dtype=np.float64) / HD))
    freqs = np.arange(S, dtype=np.float64)[None, :] * theta[:, None]   # [32, S]
    c4h = np.tile(np.cos(freqs).astype(np.float32), (4, 1))
    s4h = np.tile(np.sin(freqs).astype(np.float32), (4, 1))

    p2 = np.zeros((128, 128), dtype=np.float32)
    for p in list(range(0, 32)) + list(range(64, 96)):
        p2[p + 32, p] = -1.0
    for p in list(range(32, 64)) + list(range(96, 128)):
        p2[p - 32, p] = 1.0

    ident = np.eye(64, dtype=np.float32)
    ones32 = np.ones((128, (R // KB_W) * 64), dtype=np.float32)

    masks = np.zeros((128, 4, HPC * QS_W), dtype=np.float32)
    for t in range(4):
        m = (np.arange(QS_W)[None, :] >= (t * 128 + np.arange(128))[:, None])
        masks[:, t, :] = np.tile(m.astype(np.float32), (1, HPC))

    # Wo full, tiled for stage 3: wo_t[p, o, d, c] = Wo[d*128+p, o*128+c]
    wo_t = np.ascontiguousarray(
        Wo.astype(np.float32).reshape(OB, 128, OB, 128).transpose(1, 2, 0, 3))
    bo_t = np.ascontiguousarray(bo.astype(np.float32).reshape(OB, 128).T)

    in_maps = []
    for c in range(NC):
        wq_c = np.empty((D, 256), dtype=np.float32)
        for cb in range(2):
            for u in range(2):
                h = 4 * c + 2 * cb + u
                wq_c[:, cb * 128 + u * 64: cb * 128 + (u + 1) * 64] = Wq[:, h * 64 + perm]
        wq_t = np.ascontiguousarray(
            wq_c.reshape(D // 128, 128, 256).transpose(1, 0, 2))
        wkv_c = np.empty((D, 128), dtype=np.float32)
        wkv_c[:, 0:64] = Wk[:, c * 64 + perm]
        wkv_c[:, 64:128] = Wv[:, c * 64: (c + 1) * 64]
        wkv_t = np.ascontiguousarray(
            wkv_c.reshape(D // 128, 128, 128).transpose(1, 0, 2))
        bv_c = bv[c * 64:(c + 1) * 64].astype(np.float32).reshape(HD, 1)
        in_maps.append({
            "xta": xta, "xtb": xtb, "wq": wq_t, "wkv": wkv_t, "wo": wo_t,
            "bv": bv_c, "bo": bo_t, "c4h": c4h, "s4h": s4h,
            "p2": p2, "ident": ident, "masks": masks, "ones32": ones32,
        })
    return in_maps


def _run(in_maps, trace=False):
    if "nc" not in _CACHE:
        _CACHE["nc"] = _build()
    try:
        return run_bass_kernel_spmd(_CACHE["nc"], in_maps,
                                    core_ids=list(range(NC)), trace=trace)
    except Exception:
        # transient device wedge happens occasionally; one retry clears it
        return run_bass_kernel_spmd(_CACHE["nc"], in_maps,
                                    core_ids=list(range(NC)), trace=trace)


def _assemble(res):
    # core c's y_sh is yT for rows [512c, 512(c+1)) of the flattened (B*S) dim
    y = np.concatenate([res.results[c]["y_sh"].T for c in range(NC)], axis=0)
    return np.ascontiguousarray(y).reshape(B, S, D).astype(np.float32)


def kernel(x, Wq, Wk, Wv, bv, Wo, bo, mask):
    """Full inputs -> full output (B, S, D). `mask` is the causal tril mask
    from setup_inputs; causality is hardcoded so it is not shipped to device."""
    in_maps = _host_prep(np.asarray(x), np.asarray(Wq), np.asarray(Wk),
                         np.asarray(Wv), np.asarray(bv), np.asarray(Wo),
                         np.asarray(bo))
    res = _run(in_maps, trace=False)
    return _assemble(res)


def kernel_timed(x, Wq, Wk, Wv, bv, Wo, bo, mask):
    """Like kernel() but with NTFF tracing; returns (y, exec_time_ns)."""
    in_maps = _host_prep(np.asarray(x), np.asarray(Wq), np.asarray(Wk),
                         np.asarray(Wv), np.asarray(bv), np.asarray(Wo),
                         np.asarray(bo))
    res = _run(in_maps, trace=True)
    _CACHE["last_res"] = res
    return _assemble(res), res.exec_time_ns


# revision 24
# speedup vs baseline: 1.0736x; 1.0736x over previous
"""Trainium2 Bass kernel for causal GQA multi-head attention (nn_MHA_79362405695575).

Full (unsharded) inputs -> full output. Internally: tensor-parallel over heads
across 8 NeuronCores for projections+attention. Core c owns q-heads [4c,4c+4)
and kv-head c. After attention, a single AllToAll redistributes the attention
outputs from head-sharded to row-sharded (4.2 MB instead of ReduceScattering
33.5 MB of out-proj partials); each core then computes its 512 output rows
against the full (streamed) Wo with no further collective.

Reference semantics (fp32):
  q = x@Wq; k = x@Wk; v = x@Wv + bv           (B=2, S=2048, D=2048)
  q,k := interleaved RoPE(base 10000, hd=64)
  scores = q k^T / 8 (causal), attn = softmax
  out = attn @ v;  y = out @ Wo + bo

All matmuls run as float32r (TF32-class, ~2e-4 rel err, full PE rate).
Everything on-chip is transposed: qT/kT/vT [dim, row] layouts so no PE
transposes are needed anywhere in attention. Softmax is max-free (scores are
provably small) and denominators ride along the AV matmul as a 65th column
of v. Inputs arrive pre-tiled from the host so every DMA is a few large
contiguous transfers.
"""

import numpy as np

import concourse.bass as bass
import concourse.tile as tile
from concourse import bacc, mybir
from concourse.bass_utils import run_bass_kernel_spmd

# ---- problem constants (hardcoded; kernel.py must be self-contained) ----
B, S, D = 2, 2048, 2048
NH, NKV, HD = 32, 8, 64
ROPE_BASE = 10000.0
NC = 8                    # cores
HPC = NH // NC            # q heads per core = 4
R = B * S                 # 4096 rows
RS_N = 8                  # projection row spans
RS_W = R // RS_N          # 512 rows per span
QS_W = 512                # attention q-span width
QS_N = S // QS_W          # 4 q spans per batch
KB_W = 128                # k block width
NKB = S // KB_W           # 16 k blocks per batch
OB = D // 128             # 16 out-proj column blocks

F32 = mybir.dt.float32
F32R = mybir.dt.float32r
BF16 = mybir.dt.bfloat16

_CACHE = {}


def _build():
    nc = bacc.Bacc("TRN2", target_bir_lowering=False, debug=False, num_devices=NC)

    # ---- DRAM I/O (pre-tiled on host) ----
    xta = nc.dram_tensor("xta", [RS_N, 128, 8, RS_W], F32R, kind="ExternalInput").ap()
    xtb = nc.dram_tensor("xtb", [RS_N, 128, 8, RS_W], F32R, kind="ExternalInput").ap()
    wq = nc.dram_tensor("wq", [128, D // 128, 256], F32R, kind="ExternalInput").ap()
    wkv = nc.dram_tensor("wkv", [128, D // 128, 128], F32R, kind="ExternalInput").ap()
    wo = nc.dram_tensor("wo", [128, OB, OB, 128], BF16, kind="ExternalInput").ap()
    bv_in = nc.dram_tensor("bv", [HD, 1], F32, kind="ExternalInput").ap()
    bo_in = nc.dram_tensor("bo", [128, OB], F32, kind="ExternalInput").ap()
    c4h = nc.dram_tensor("c4h", [128, S], F32, kind="ExternalInput").ap()
    s4h = nc.dram_tensor("s4h", [128, S], F32, kind="ExternalInput").ap()
    p2 = nc.dram_tensor("p2", [128, 128], F32R, kind="ExternalInput").ap()
    ident = nc.dram_tensor("ident", [64, 64], F32R, kind="ExternalInput").ap()
    masks = nc.dram_tensor("masks", [128, 4, 2 * QS_W], F32R, kind="ExternalInput").ap()
    ones32 = nc.dram_tensor("ones32", [128, (R // KB_W) * 64], F32R,
                            kind="ExternalInput").ap()
    y_sh = nc.dram_tensor("y_sh", [D, RS_W], F32, kind="ExternalOutput").ap()

    DMA = nc.sync

    with tile.TileContext(nc) as tc:
        with (
            tc.tile_pool(name="persist", bufs=1) as pp,
            tc.tile_pool(name="dram", bufs=1, space="DRAM") as dram,
        ):
            # ---- persistent SBUF (whole kernel) ----
            qrT = [pp.tile([128, R], F32R, tag=f"qrT{t}", name=f"qrT{t}") for t in range(2)]
            krT = pp.tile([128, R], F32R, tag="krT")
            # v_aug cols 0:64 = ones (so AV materializes the softmax
            # denominator on partitions 0:64 of pav), cols 64:128 = v.
            v_aug = pp.tile([128, R // KB_W, 128], F32R, tag="vaug")
            p2_sb = pp.tile([128, 128], F32R, tag="p2")
            id_sb = pp.tile([64, 64], F32R, tag="ident")
            bv_sb = pp.tile([HD, 1], F32, tag="bv")
            bo_sb = pp.tile([128, OB], F32, tag="bo")

            DMA.dma_start(out=p2_sb[:], in_=p2[:])
            DMA.dma_start(out=id_sb[:], in_=ident[:])
            DMA.dma_start(out=bv_sb[:], in_=bv_in[:])
            DMA.dma_start(out=bo_sb[:], in_=bo_in[:])
            DMA.dma_start(out=v_aug[:, :, 0:64],
                          in_=ones32.rearrange("p (j o) -> p j o", o=64))

            # AllToAll buffers, one per head-pair pass: shard j of a2a_in[g]
            # = this core's pair-g tile for row-block j; after the A2A,
            # a2a_out[g][j] = core j's pair g for THIS core's row-block.
            # Two collectives so the first runs under the second g-pass.
            a2a_in = [dram.tile([NC, 128, RS_W], BF16, name=f"a2ai{g}")
                      for g in range(2)]
            a2a_out = [dram.tile([NC, 128, RS_W], BF16, name=f"a2ao{g}")
                       for g in range(2)]

            # masks prefetched into persistent SBUF at t=0 so stage 2
            # does not wait on stage-1 pool space being freed.
            mask_sb = pp.tile([128, 4, 2 * QS_W], F32R, tag="masks")
            DMA.dma_start(out=mask_sb[:], in_=masks[:])

            # ================= stage 1: projections + RoPE =================
            with (
                tc.tile_pool(name="w1p", bufs=1) as w1p,
                tc.tile_pool(name="xtpa", bufs=2) as xtpa,
                tc.tile_pool(name="xtpb", bufs=2) as xtpb,
                tc.tile_pool(name="ropet", bufs=2) as ropet,
                tc.tile_pool(name="vstg", bufs=2) as vstg,
                tc.tile_pool(name="ps_q", bufs=2, space="PSUM") as ps_q,
                tc.tile_pool(name="ps_kv", bufs=2, space="PSUM") as ps_kv,
                tc.tile_pool(name="ps_sw", bufs=2, space="PSUM") as ps_sw,
                tc.tile_pool(name="ps_vt", bufs=2, space="PSUM") as ps_vt,
            ):
                wq_sb = w1p.tile([128, D // 128, 256], F32R, tag="wq")
                wkv_sb = w1p.tile([128, D // 128, 128], F32R, tag="wkv")
                c4_sb = w1p.tile([128, S], F32, tag="c4")
                s4_sb = w1p.tile([128, S], F32, tag="s4")
                DMA.dma_start(out=wq_sb[:], in_=wq[:])
                DMA.dma_start(out=wkv_sb[:], in_=wkv[:])
                DMA.dma_start(out=c4_sb[:], in_=c4h[:])
                DMA.dma_start(out=s4_sb[:], in_=s4h[:])
                SPB = RS_N // B          # spans per batch
                for rs in range(RS_N):
                    rsl = slice(rs * RS_W, (rs + 1) * RS_W)
                    ssl = slice((rs % SPB) * RS_W, (rs % SPB + 1) * RS_W)
                    xa = xtpa.tile([128, 8, RS_W], F32R, tag="xa")
                    xb = xtpb.tile([128, 8, RS_W], F32R, tag="xb")
                    DMA.dma_start(out=xa[:], in_=xta[rs])
                    DMA.dma_start(out=xb[:], in_=xtb[rs])

                    def xt(kb):
                        return xa[:, kb, :] if kb < 8 else xb[:, kb - 8, :]

                    # -- q projection: 2 colblocks (2 heads each) --
                    for cb in range(2):
                        pq = ps_q.tile([128, RS_W], F32, tag="pq")
                        for kb in range(D // 128):
                            nc.tensor.matmul(pq[:], wq_sb[:, kb, cb * 128:(cb + 1) * 128],
                                             xt(kb),
                                             start=(kb == 0), stop=(kb == D // 128 - 1))
                        # RoPE: qr = pq*C + P2.T @ (pq*S)
                        st = ropet.tile([128, RS_W], F32R, tag="st")
                        nc.vector.tensor_tensor(out=st[:], in0=pq[:], in1=s4_sb[:, ssl],
                                                op=mybir.AluOpType.mult)
                        sw = ps_sw.tile([128, RS_W], F32, tag="sw")
                        nc.tensor.matmul(sw[:], p2_sb[:], st[:], start=True, stop=True)
                        ct = ropet.tile([128, RS_W], F32, tag="ct")
                        nc.vector.tensor_tensor(out=ct[:], in0=pq[:], in1=c4_sb[:, ssl],
                                                op=mybir.AluOpType.mult)
                        nc.vector.tensor_tensor(out=qrT[cb][:, rsl], in0=ct[:], in1=sw[:],
                                                op=mybir.AluOpType.add)

                    # -- kv projection: cols 0:64 = kT(perm), 64:128 = vT --
                    pkv = ps_kv.tile([128, RS_W], F32, tag="pkv")
                    for kb in range(D // 128):
                        nc.tensor.matmul(pkv[:], wkv_sb[:, kb, :], xt(kb),
                                         start=(kb == 0), stop=(kb == D // 128 - 1))
                    # k RoPE (partitions 0:64), duplicated into krT[0:64] and [64:128]
                    stk = ropet.tile([64, RS_W], F32R, tag="stk")
                    nc.vector.tensor_tensor(out=stk[:], in0=pkv[0:64, :],
                                            in1=s4_sb[0:64, ssl], op=mybir.AluOpType.mult)
                    swk = ps_sw.tile([64, RS_W], F32, tag="sw")
                    nc.tensor.matmul(swk[:], p2_sb[0:64, 0:64], stk[:], start=True, stop=True)
                    ctk = ropet.tile([64, RS_W], F32, tag="ctk")
                    nc.vector.tensor_tensor(out=ctk[:], in0=pkv[0:64, :],
                                            in1=c4_sb[0:64, ssl], op=mybir.AluOpType.mult)
                    nc.vector.tensor_tensor(out=krT[0:64, rsl], in0=ctk[:], in1=swk[:],
                                            op=mybir.AluOpType.add)
                    nc.vector.tensor_tensor(out=krT[64:128, rsl], in0=ctk[:], in1=swk[:],
                                            op=mybir.AluOpType.add)

                    # v: bias add then transpose [64,128] -> [128,64] blocks
                    vst = vstg.tile([64, RS_W], F32R, tag="vst")
                    nc.scalar.activation(out=vst[:], in_=pkv[64:128, :],
                                         func=mybir.ActivationFunctionType.Identity,
                                         bias=bv_sb[:], scale=1.0)
                    for j in range(RS_W // KB_W):
                        pv = ps_vt.tile([128, 64], F32R, tag="pv")
                        nc.tensor.transpose(pv[:], vst[:, j * 128:(j + 1) * 128], id_sb[:])
                        nc.vector.tensor_copy(
                            out=v_aug[:, rs * (RS_W // KB_W) + j, 64:128], in_=pv[:])

            # ============ stage 2: attention -> per-span A2A shards ============
            # g (head-pair) is the OUTER loop: pass g's A2A is triggered as
            # soon as that pass ends, so the g=0 exchange runs underneath the
            # g=1 compute pass and only the g=1 exchange is exposed.
            with (
                tc.tile_pool(name="ptp", bufs=5) as ptp,
                tc.tile_pool(name="normp", bufs=2) as normp,
                tc.tile_pool(name="otp", bufs=4) as otp,
                tc.tile_pool(name="ps_s", bufs=2, space="PSUM") as ps_s,
                tc.tile_pool(name="ps_av", bufs=2, space="PSUM") as ps_av,
            ):
                for g in range(2):
                    for b in range(B):
                        for qs in range(QS_N):
                            jrow = SPB * b + qs
                            n_kb = 4 * (qs + 1)
                            qsl = slice(b * S + qs * QS_W, b * S + (qs + 1) * QS_W)
                            # software pipeline, scalar-paced: the 2 score MMs
                            # write a double-buffered [128,1024] PSUM tile so
                            # scores(kb+1) overlap exp(kb); the AV MMs for
                            # (kb-1) are issued after scores(kb) and run under
                            # exp(kb). pav is allocated at first AV use so the
                            # previous span's drain doesn't gate scores(kb=0).
                            pav = None
                            pts = [None] * n_kb
                            for kb in range(n_kb + 1):
                                if kb < n_kb:
                                    kbl = slice(b * S + kb * KB_W,
                                                b * S + (kb + 1) * KB_W)
                                    dlt = kb - 4 * qs
                                    pss = ps_s.tile([128, 2 * QS_W], F32,
                                                    tag="pss")
                                    for u in range(2):
                                        nc.tensor.matmul(
                                            pss[:, u * QS_W:(u + 1) * QS_W],
                                            krT[u * 64:(u + 1) * 64, kbl],
                                            qrT[g][u * 64:(u + 1) * 64, qsl],
                                            start=True, stop=True)
                                    pt = ptp.tile([128, 2 * QS_W], F32R,
                                                  tag="pt")
                                    nc.scalar.activation(
                                        out=pt[:], in_=pss[:],
                                        func=mybir.ActivationFunctionType.Exp,
                                        scale=float(HD) ** -0.5)
                                    if dlt >= 0:
                                        nc.vector.tensor_tensor(
                                            out=pt[:], in0=pt[:],
                                            in1=mask_sb[:, dlt, 0:2 * QS_W],
                                            op=mybir.AluOpType.mult)
                                    pts[kb] = pt
                                if kb > 0:
                                    if pav is None:
                                        pav = ps_av.tile([128, 2 * QS_W], F32,
                                                         tag="pav")
                                    pt1 = pts[kb - 1]
                                    for u in range(2):
                                        nc.tensor.matmul(
                                            pav[:, u * QS_W:(u + 1) * QS_W],
                                            v_aug[:, b * NKB + kb - 1, :],
                                            pt1[:, u * QS_W:(u + 1) * QS_W],
                                            start=(kb == 1), stop=(kb == n_kb))
                            # normalize: pav rows 0:64 = denominator (dup x64),
                            # rows 64:128 = out values. All on vector.
                            rbs = normp.tile([64, 2 * QS_W], F32, tag="rbs")
                            nc.vector.reciprocal_approx_fast(
                                out=rbs[:], in_=pav[0:64, :])
                            ot = otp.tile([128, QS_W], BF16, tag="ot")
                            for u in range(2):
                                nc.vector.tensor_tensor(
                                    out=ot[u * 64:(u + 1) * 64, :],
                                    in0=pav[64:128, u * QS_W:(u + 1) * QS_W],
                                    in1=rbs[:, u * QS_W:(u + 1) * QS_W],
                                    op=mybir.AluOpType.mult)
                            DMA.dma_start(out=a2a_in[g][jrow], in_=ot[:])
                    # pass-g exchange (g=0's overlaps the g=1 pass)
                    nc.gpsimd.collective_compute(
                        "AllToAll", mybir.AluOpType.bypass,
                        replica_groups=[list(range(NC))],
                        ins=[a2a_in[g][:]], outs=[a2a_out[g][:]],
                    )

            # ---- stage 3: local out-projection over full Wo ----
            with (
                tc.tile_pool(name="w3p", bufs=1) as w3p,
                tc.tile_pool(name="wop", bufs=2) as wop,
                tc.tile_pool(name="ystg", bufs=4) as ystg,
                tc.tile_pool(name="ps_y", bufs=3, space="PSUM") as ps_y,
                tc.tile_pool(name="ps_kw", bufs=1, space="PSUM") as ps_kw,
            ):
                # keep the PE's HAM clock warm while the A2A runs: ~400
                # dependency-free matmuls on the tensor queue fill the
                # otherwise-idle collective window so stage 3 starts at
                # full clock instead of 1.2 GHz.
                kw = ps_kw.tile([128, 2 * KB_W], F32, tag="kw")
                for _ in range(300):
                    nc.tensor.matmul(kw[:], p2_sb[:],
                                     qrT[0][:, 0:2 * KB_W],
                                     start=True, stop=True)
                of_sb = w3p.tile([128, OB, RS_W], BF16, tag="of")
                for dd in range(OB):
                    DMA.dma_start(out=of_sb[:, dd, :], in_=a2a_out[dd % 2][dd // 2])
                for o in range(OB):
                    wo_sb = wop.tile([128, OB, 128], BF16, tag="wo")
                    DMA.dma_start(out=wo_sb[:], in_=wo[:, o])
                    py = ps_y.tile([128, RS_W], F32, tag="py")
                    for dd in range(OB):
                        nc.tensor.matmul(py[:], wo_sb[:, dd, :], of_sb[:, dd, :],
                                         start=(dd == 0), stop=(dd == OB - 1))
                    ys = ystg.tile([128, RS_W], F32, tag="ys")
                    nc.scalar.activation(out=ys[:], in_=py[:],
                                         func=mybir.ActivationFunctionType.Identity,
                                         bias=bo_sb[:, o:o + 1], scale=1.0)
                    DMA.dma_start(out=y_sh[o * 128:(o + 1) * 128, :], in_=ys[:])

    nc.finalize()
    return nc


def _rope_perm():
    return np.concatenate([np.arange(0, HD, 2), np.arange(1, HD, 2)])


def _host_prep(x, Wq, Wk, Wv, bv, Wo, bo):
    """Build per-core input maps (inputs pre-tiled to SBUF layouts)."""
    perm = _rope_perm()

    # x tiled: A[kb, p, r] = x[r, kb*128+p];  xta = kb 0..7, xtb = kb 8..15
    A = np.ascontiguousarray(x.reshape(R, D).T).reshape(D // 128, 128, R)
    xta = np.ascontiguousarray(
        A[0:8].reshape(8, 128, RS_N, RS_W).transpose(2, 1, 0, 3)).astype(np.float32)
    xtb = np.ascontiguousarray(
        A[8:16].reshape(8, 128, RS_N, RS_W).transpose(2, 1, 0, 3)).astype(np.float32)

    theta = (1.0 / ROPE_BASE ** (np.arange(0, HD, 2, dtype=np.float64) / HD))
    freqs = np.arange(S, dtype=np.float64)[None, :] * theta[:, None]   # [32, S]
    c4h = np.tile(np.cos(freqs).astype(np.float32), (4, 1))
    s4h = np.tile(np.sin(freqs).astype(np.float32), (4, 1))

    p2 = np.zeros((128, 128), dtype=np.float32)
    for p in list(range(0, 32)) + list(range(64, 96)):
        p2[p + 32, p] = -1.0
    for p in list(range(32, 64)) + list(range(96, 128)):
        p2[p - 32, p] = 1.0

    ident = np.eye(64, dtype=np.float32)
    ones32 = np.ones((128, (R // KB_W) * 64), dtype=np.float32)

    masks = np.zeros((128, 4, 2 * QS_W), dtype=np.float32)
    for t in range(4):
        m = (np.arange(QS_W)[None, :] >= (t * 128 + np.arange(128))[:, None])
        masks[:, t, :] = np.tile(m.astype(np.float32), (1, 2))

    # Wo full, tiled for stage 3: wo_t[p, o, d, c] = Wo[d*128+p, o*128+c]
    wo_t = np.ascontiguousarray(
        Wo.astype(np.float32).reshape(OB, 128, OB, 128).transpose(1, 2, 0, 3)
    ).astype(mybir.dt.np(mybir.dt.bfloat16))
    bo_t = np.ascontiguousarray(bo.astype(np.float32).reshape(OB, 128).T)

    in_maps = []
    for c in range(NC):
        wq_c = np.empty((D, 256), dtype=np.float32)
        for cb in range(2):
            for u in range(2):
                h = 4 * c + 2 * cb + u
                wq_c[:, cb * 128 + u * 64: cb * 128 + (u + 1) * 64] = Wq[:, h * 64 + perm]
        wq_t = np.ascontiguousarray(
            wq_c.reshape(D // 128, 128, 256).transpose(1, 0, 2))
        wkv_c = np.empty((D, 128), dtype=np.float32)
        wkv_c[:, 0:64] = Wk[:, c * 64 + perm]
        wkv_c[:, 64:128] = Wv[:, c * 64: (c + 1) * 64]
        wkv_t = np.ascontiguousarray(
            wkv_c.reshape(D // 128, 128, 128).transpose(1, 0, 2))
        bv_c = bv[c * 64:(c + 1) * 64].astype(np.float32).reshape(HD, 1)
        in_maps.append({
            "xta": xta, "xtb": xtb, "wq": wq_t, "wkv": wkv_t, "wo": wo_t,
            "bv": bv_c, "bo": bo_t, "c4h": c4h, "s4h": s4h,
            "p2": p2, "ident": ident, "masks": masks, "ones32": ones32,
        })
    return in_maps


def _run(in_maps, trace=False):
    if "nc" not in _CACHE:
        _CACHE["nc"] = _build()
    try:
        return run_bass_kernel_spmd(_CACHE["nc"], in_maps,
                                    core_ids=list(range(NC)), trace=trace)
    except Exception:
        # transient device wedge happens occasionally; one retry clears it
        return run_bass_kernel_spmd(_CACHE["nc"], in_maps,
                                    core_ids=list(range(NC)), trace=trace)


def _assemble(res):
    # core c's y_sh is yT for rows [512c, 512(c+1)) of the flattened (B*S) dim
    y = np.concatenate([res.results[c]["y_sh"].T for c in range(NC)], axis=0)
    return np.ascontiguousarray(y).reshape(B, S, D).astype(np.float32)


def kernel(x, Wq, Wk, Wv, bv, Wo, bo, mask):
    """Full inputs -> full output (B, S, D). `mask` is the causal tril mask
    from setup_inputs; causality is hardcoded so it is not shipped to device."""
    in_maps = _host_prep(np.asarray(x), np.asarray(Wq), np.asarray(Wk),
                         np.asarray(Wv), np.asarray(bv), np.asarray(Wo),
                         np.asarray(bo))
    res = _run(in_maps, trace=False)
    return _assemble(res)


def kernel_timed(x, Wq, Wk, Wv, bv, Wo, bo, mask):
    """Like kernel() but with NTFF tracing; returns (y, exec_time_ns)."""
    in_maps = _host_prep(np.asarray(x), np.asarray(Wq), np.asarray(Wk),
                         np.asarray(Wv), np.asarray(bv), np.asarray(Wo),
                         np.asarray(bo))
    res = _run(in_maps, trace=True)
    _CACHE["last_res"] = res
    return _assemble(res), res.exec_time_ns


# revision 28
# speedup vs baseline: 1.1121x; 1.0359x over previous
"""Trainium2 Bass kernel for causal GQA multi-head attention (nn_MHA_79362405695575).

Full (unsharded) inputs -> full output. Internally: tensor-parallel over heads
across 8 NeuronCores for projections+attention. Core c owns q-heads [4c,4c+4)
and kv-head c. After attention, a single AllToAll redistributes the attention
outputs from head-sharded to row-sharded (4.2 MB instead of ReduceScattering
33.5 MB of out-proj partials); each core then computes its 512 output rows
against the full (streamed) Wo with no further collective.

Reference semantics (fp32):
  q = x@Wq; k = x@Wk; v = x@Wv + bv           (B=2, S=2048, D=2048)
  q,k := interleaved RoPE(base 10000, hd=64)
  scores = q k^T / 8 (causal), attn = softmax
  out = attn @ v;  y = out @ Wo + bo

All matmuls run as float32r (TF32-class, ~2e-4 rel err, full PE rate).
Everything on-chip is transposed: qT/kT/vT [dim, row] layouts so no PE
transposes are needed anywhere in attention. Softmax is max-free (scores are
provably small) and denominators ride along the AV matmul as a 65th column
of v. Inputs arrive pre-tiled from the host so every DMA is a few large
contiguous transfers.
"""

import numpy as np

import concourse.bass as bass
import concourse.tile as tile
from concourse import bacc, mybir
from concourse.bass_utils import run_bass_kernel_spmd

# ---- problem constants (hardcoded; kernel.py must be self-contained) ----
B, S, D = 2, 2048, 2048
NH, NKV, HD = 32, 8, 64
ROPE_BASE = 10000.0
NC = 8                    # cores
HPC = NH // NC            # q heads per core = 4
R = B * S                 # 4096 rows
RS_N = 8                  # projection row spans
RS_W = R // RS_N          # 512 rows per span
QS_W = 512                # attention q-span width
QS_N = S // QS_W          # 4 q spans per batch
KB_W = 128                # k block width
NKB = S // KB_W           # 16 k blocks per batch
OB = D // 128             # 16 out-proj column blocks

F32 = mybir.dt.float32
F32R = mybir.dt.float32r
BF16 = mybir.dt.bfloat16

_CACHE = {}


def _build():
    nc = bacc.Bacc("TRN2", target_bir_lowering=False, debug=False, num_devices=NC)

    # ---- DRAM I/O (pre-tiled on host) ----
    xta = nc.dram_tensor("xta", [RS_N, 128, 8, RS_W], F32R, kind="ExternalInput").ap()
    xtb = nc.dram_tensor("xtb", [RS_N, 128, 8, RS_W], F32R, kind="ExternalInput").ap()
    wq = nc.dram_tensor("wq", [128, D // 128, 256], F32R, kind="ExternalInput").ap()
    wkv = nc.dram_tensor("wkv", [128, D // 128, 128], F32R, kind="ExternalInput").ap()
    wo = nc.dram_tensor("wo", [128, OB, OB, 128], BF16, kind="ExternalInput").ap()
    bv_in = nc.dram_tensor("bv", [HD, 1], F32, kind="ExternalInput").ap()
    bo_in = nc.dram_tensor("bo", [128, OB], F32, kind="ExternalInput").ap()
    c4h = nc.dram_tensor("c4h", [128, S], F32, kind="ExternalInput").ap()
    s4h = nc.dram_tensor("s4h", [128, S], F32, kind="ExternalInput").ap()
    p2 = nc.dram_tensor("p2", [128, 128], F32R, kind="ExternalInput").ap()
    ident = nc.dram_tensor("ident", [64, 64], F32R, kind="ExternalInput").ap()
    masks = nc.dram_tensor("masks", [128, 4, 2 * QS_W], F32R, kind="ExternalInput").ap()
    ones32 = nc.dram_tensor("ones32", [128, (R // KB_W) * 64], F32R,
                            kind="ExternalInput").ap()
    y_sh = nc.dram_tensor("y_sh", [D, RS_W], F32, kind="ExternalOutput").ap()

    DMA = nc.sync

    with tile.TileContext(nc) as tc:
        with (
            tc.tile_pool(name="persist", bufs=1) as pp,
            tc.tile_pool(name="dram", bufs=1, space="DRAM") as dram,
        ):
            # ---- persistent SBUF (whole kernel) ----
            qrT = [pp.tile([128, R], F32R, tag=f"qrT{t}", name=f"qrT{t}") for t in range(2)]
            krT = pp.tile([128, R], F32R, tag="krT")
            # v_aug cols 0:64 = ones (so AV materializes the softmax
            # denominator on partitions 0:64 of pav), cols 64:128 = v.
            v_aug = pp.tile([128, R // KB_W, 128], F32R, tag="vaug")
            p2_sb = pp.tile([128, 128], F32R, tag="p2")
            id_sb = pp.tile([64, 64], F32R, tag="ident")
            bv_sb = pp.tile([HD, 1], F32, tag="bv")
            bo_sb = pp.tile([128, OB], F32, tag="bo")

            DMA.dma_start(out=p2_sb[:], in_=p2[:])
            DMA.dma_start(out=id_sb[:], in_=ident[:])
            DMA.dma_start(out=bv_sb[:], in_=bv_in[:])
            DMA.dma_start(out=bo_sb[:], in_=bo_in[:])
            DMA.dma_start(out=v_aug[:, :, 0:64],
                          in_=ones32.rearrange("p (j o) -> p j o", o=64))

            # AllToAll buffers: shard j of a2a_in = this core's 2 head-pair
            # tiles for row-block j; after A2A, a2a_out[j, g] = core j's
            # head-pair g for THIS core's row-block.
            a2a_in = dram.tile([NC, 2, 128, RS_W], BF16)
            a2a_out = dram.tile([NC, 2, 128, RS_W], BF16)

            # masks prefetched into persistent SBUF at t=0 so stage 2
            # does not wait on stage-1 pool space being freed.
            mask_sb = pp.tile([128, 4, 2 * QS_W], F32R, tag="masks")
            DMA.dma_start(out=mask_sb[:], in_=masks[:])

            # ================= stage 1: projections + RoPE =================
            with (
                tc.tile_pool(name="w1p", bufs=1) as w1p,
                tc.tile_pool(name="xtpa", bufs=2) as xtpa,
                tc.tile_pool(name="xtpb", bufs=2) as xtpb,
                tc.tile_pool(name="ropet", bufs=2) as ropet,
                tc.tile_pool(name="vstg", bufs=2) as vstg,
                tc.tile_pool(name="ps_q", bufs=2, space="PSUM") as ps_q,
                tc.tile_pool(name="ps_kv", bufs=2, space="PSUM") as ps_kv,
                tc.tile_pool(name="ps_sw", bufs=2, space="PSUM") as ps_sw,
                tc.tile_pool(name="ps_vt", bufs=2, space="PSUM") as ps_vt,
            ):
                wq_sb = w1p.tile([128, D // 128, 256], F32R, tag="wq")
                wkv_sb = w1p.tile([128, D // 128, 128], F32R, tag="wkv")
                c4_sb = w1p.tile([128, S], F32, tag="c4")
                s4_sb = w1p.tile([128, S], F32, tag="s4")
                DMA.dma_start(out=wq_sb[:], in_=wq[:])
                DMA.dma_start(out=wkv_sb[:], in_=wkv[:])
                DMA.dma_start(out=c4_sb[:], in_=c4h[:])
                DMA.dma_start(out=s4_sb[:], in_=s4h[:])
                SPB = RS_N // B          # spans per batch
                for rs in range(RS_N):
                    rsl = slice(rs * RS_W, (rs + 1) * RS_W)
                    ssl = slice((rs % SPB) * RS_W, (rs % SPB + 1) * RS_W)
                    xa = xtpa.tile([128, 8, RS_W], F32R, tag="xa")
                    xb = xtpb.tile([128, 8, RS_W], F32R, tag="xb")
                    DMA.dma_start(out=xa[:], in_=xta[rs])
                    DMA.dma_start(out=xb[:], in_=xtb[rs])

                    def xt(kb):
                        return xa[:, kb, :] if kb < 8 else xb[:, kb - 8, :]

                    # -- q projection: 2 colblocks (2 heads each) --
                    for cb in range(2):
                        pq = ps_q.tile([128, RS_W], F32, tag="pq")
                        for kb in range(D // 128):
                            nc.tensor.matmul(pq[:], wq_sb[:, kb, cb * 128:(cb + 1) * 128],
                                             xt(kb),
                                             start=(kb == 0), stop=(kb == D // 128 - 1))
                        # RoPE: qr = pq*C + P2.T @ (pq*S)
                        st = ropet.tile([128, RS_W], F32R, tag="st")
                        nc.vector.tensor_tensor(out=st[:], in0=pq[:], in1=s4_sb[:, ssl],
                                                op=mybir.AluOpType.mult)
                        sw = ps_sw.tile([128, RS_W], F32, tag="sw")
                        nc.tensor.matmul(sw[:], p2_sb[:], st[:], start=True, stop=True)
                        ct = ropet.tile([128, RS_W], F32, tag="ct")
                        nc.vector.tensor_tensor(out=ct[:], in0=pq[:], in1=c4_sb[:, ssl],
                                                op=mybir.AluOpType.mult)
                        nc.vector.tensor_tensor(out=qrT[cb][:, rsl], in0=ct[:], in1=sw[:],
                                                op=mybir.AluOpType.add)

                    # -- kv projection: cols 0:64 = kT(perm), 64:128 = vT --
                    pkv = ps_kv.tile([128, RS_W], F32, tag="pkv")
                    for kb in range(D // 128):
                        nc.tensor.matmul(pkv[:], wkv_sb[:, kb, :], xt(kb),
                                         start=(kb == 0), stop=(kb == D // 128 - 1))
                    # k RoPE (partitions 0:64), duplicated into krT[0:64] and [64:128]
                    stk = ropet.tile([64, RS_W], F32R, tag="stk")
                    nc.vector.tensor_tensor(out=stk[:], in0=pkv[0:64, :],
                                            in1=s4_sb[0:64, ssl], op=mybir.AluOpType.mult)
                    swk = ps_sw.tile([64, RS_W], F32, tag="sw")
                    nc.tensor.matmul(swk[:], p2_sb[0:64, 0:64], stk[:], start=True, stop=True)
                    ctk = ropet.tile([64, RS_W], F32, tag="ctk")
                    nc.vector.tensor_tensor(out=ctk[:], in0=pkv[0:64, :],
                                            in1=c4_sb[0:64, ssl], op=mybir.AluOpType.mult)
                    nc.vector.tensor_tensor(out=krT[0:64, rsl], in0=ctk[:], in1=swk[:],
                                            op=mybir.AluOpType.add)
                    nc.vector.tensor_tensor(out=krT[64:128, rsl], in0=ctk[:], in1=swk[:],
                                            op=mybir.AluOpType.add)

                    # v: bias add then transpose [64,128] -> [128,64] blocks
                    vst = vstg.tile([64, RS_W], F32R, tag="vst")
                    nc.scalar.activation(out=vst[:], in_=pkv[64:128, :],
                                         func=mybir.ActivationFunctionType.Identity,
                                         bias=bv_sb[:], scale=1.0)
                    for j in range(RS_W // KB_W):
                        pv = ps_vt.tile([128, 64], F32R, tag="pv")
                        nc.tensor.transpose(pv[:], vst[:, j * 128:(j + 1) * 128], id_sb[:])
                        nc.vector.tensor_copy(
                            out=v_aug[:, rs * (RS_W // KB_W) + j, 64:128], in_=pv[:])

            # ============ stage 2: attention -> per-span A2A shards ============
            with (
                tc.tile_pool(name="ptp", bufs=5) as ptp,
                tc.tile_pool(name="normp", bufs=2) as normp,
                tc.tile_pool(name="otp", bufs=4) as otp,
                tc.tile_pool(name="ps_s", bufs=2, space="PSUM") as ps_s,
                tc.tile_pool(name="ps_av", bufs=2, space="PSUM") as ps_av,
            ):
                for b in range(B):
                    for qs in range(QS_N):
                        jrow = SPB * b + qs
                        n_kb = 4 * (qs + 1)
                        qsl = slice(b * S + qs * QS_W, b * S + (qs + 1) * QS_W)
                        # software pipeline, scalar-paced: per (kb, g) the 2
                        # score MMs write a double-buffered [128,1024] PSUM
                        # tile so scores(kb+1) overlap exp(kb); the AV MMs
                        # for (kb-1) are issued after scores(kb) and run
                        # under exp(kb). pav tiles are allocated at first AV
                        # use so the previous span's drain (vector) doesn't
                        # gate this span's first score matmuls.
                        pav = [None, None]
                        pts = [[None] * 2 for _ in range(n_kb)]
                        for kb in range(n_kb + 1):
                            if kb < n_kb:
                                kbl = slice(b * S + kb * KB_W,
                                            b * S + (kb + 1) * KB_W)
                                dlt = kb - 4 * qs
                                for g in range(2):
                                    pss = ps_s.tile([128, 2 * QS_W], F32,
                                                    tag="pss")
                                    for u in range(2):
                                        nc.tensor.matmul(
                                            pss[:, u * QS_W:(u + 1) * QS_W],
                                            krT[u * 64:(u + 1) * 64, kbl],
                                            qrT[g][u * 64:(u + 1) * 64, qsl],
                                            start=True, stop=True)
                                    pt = ptp.tile([128, 2 * QS_W], F32R,
                                                  tag="pt")
                                    nc.scalar.activation(
                                        out=pt[:], in_=pss[:],
                                        func=mybir.ActivationFunctionType.Exp,
                                        scale=float(HD) ** -0.5)
                                    if dlt >= 0:
                                        nc.vector.tensor_tensor(
                                            out=pt[:], in0=pt[:],
                                            in1=mask_sb[:, dlt, 0:2 * QS_W],
                                            op=mybir.AluOpType.mult)
                                    pts[kb][g] = pt
                            if kb > 0:
                                for g in range(2):
                                    if pav[g] is None:
                                        pav[g] = ps_av.tile(
                                            [128, 2 * QS_W], F32, tag="pav",
                                            name=f"pav{b}{qs}{g}")
                                    pt1 = pts[kb - 1][g]
                                    for u in range(2):
                                        nc.tensor.matmul(
                                            pav[g][:, u * QS_W:(u + 1) * QS_W],
                                            v_aug[:, b * NKB + kb - 1, :],
                                            pt1[:, u * QS_W:(u + 1) * QS_W],
                                            start=(kb == 1), stop=(kb == n_kb))
                        # normalize: pav rows 0:64 = denominator (dup x64),
                        # rows 64:128 = out values. All on vector; the
                        # custom-DVE recip reads PSUM at partition 0 (safe).
                        for g in range(2):
                            rbs = normp.tile([64, 2 * QS_W], F32, tag="rbs")
                            nc.vector.reciprocal_approx_fast(
                                out=rbs[:], in_=pav[g][0:64, :])
                            ot = otp.tile([128, QS_W], BF16, tag="ot")
                            for u in range(2):
                                nc.vector.tensor_tensor(
                                    out=ot[u * 64:(u + 1) * 64, :],
                                    in0=pav[g][64:128, u * QS_W:(u + 1) * QS_W],
                                    in1=rbs[:, u * QS_W:(u + 1) * QS_W],
                                    op=mybir.AluOpType.mult)
                            DMA.dma_start(out=a2a_in[jrow, g], in_=ot[:])

            # ---- stage 2.5: redistribute heads -> rows ----
            nc.gpsimd.collective_compute(
                "AllToAll", mybir.AluOpType.bypass,
                replica_groups=[list(range(NC))],
                ins=[a2a_in[:]], outs=[a2a_out[:]],
            )

            # ---- stage 3: local out-projection over full Wo ----
            with (
                tc.tile_pool(name="w3p", bufs=1) as w3p,
                tc.tile_pool(name="wop", bufs=2) as wop,
                tc.tile_pool(name="ystg", bufs=4) as ystg,
                tc.tile_pool(name="ps_y", bufs=3, space="PSUM") as ps_y,
                tc.tile_pool(name="ps_kw", bufs=1, space="PSUM") as ps_kw,
            ):
                # keep the PE's HAM clock warm while the A2A runs: ~400
                # dependency-free matmuls on the tensor queue fill the
                # otherwise-idle collective window so stage 3 starts at
                # full clock instead of 1.2 GHz.
                kw = ps_kw.tile([128, 2 * KB_W], F32, tag="kw")
                for _ in range(650):
                    nc.tensor.matmul(kw[:], p2_sb[:],
                                     qrT[0][:, 0:2 * KB_W],
                                     start=True, stop=True)
                of_sb = w3p.tile([128, OB, RS_W], BF16, tag="of")
                for dd in range(OB):
                    DMA.dma_start(out=of_sb[:, dd, :], in_=a2a_out[dd // 2, dd % 2])
                for o in range(OB):
                    wo_sb = wop.tile([128, OB, 128], BF16, tag="wo")
                    DMA.dma_start(out=wo_sb[:], in_=wo[:, o])
                    py = ps_y.tile([128, RS_W], F32, tag="py")
                    for dd in range(OB):
                        nc.tensor.matmul(py[:], wo_sb[:, dd, :], of_sb[:, dd, :],
                                         start=(dd == 0), stop=(dd == OB - 1))
                    ys = ystg.tile([128, RS_W], F32, tag="ys")
                    nc.scalar.activation(out=ys[:], in_=py[:],
                                         func=mybir.ActivationFunctionType.Identity,
                                         bias=bo_sb[:, o:o + 1], scale=1.0)
                    DMA.dma_start(out=y_sh[o * 128:(o + 1) * 128, :], in_=ys[:])

    nc.finalize()
    return nc


def _rope_perm():
    return np.concatenate([np.arange(0, HD, 2), np.arange(1, HD, 2)])


def _host_prep(x, Wq, Wk, Wv, bv, Wo, bo):
    """Build per-core input maps (inputs pre-tiled to SBUF layouts)."""
    perm = _rope_perm()

    # x tiled: A[kb, p, r] = x[r, kb*128+p];  xta = kb 0..7, xtb = kb 8..15
    A = np.ascontiguousarray(x.reshape(R, D).T).reshape(D // 128, 128, R)
    xta = np.ascontiguousarray(
        A[0:8].reshape(8, 128, RS_N, RS_W).transpose(2, 1, 0, 3)).astype(np.float32)
    xtb = np.ascontiguousarray(
        A[8:16].reshape(8, 128, RS_N, RS_W).transpose(2, 1, 0, 3)).astype(np.float32)

    theta = (1.0 / ROPE_BASE ** (np.arange(0, HD, 2, dtype=np.float64) / HD))
    freqs = np.arange(S, dtype=np.float64)[None, :] * theta[:, None]   # [32, S]
    c4h = np.tile(np.cos(freqs).astype(np.float32), (4, 1))
    s4h = np.tile(np.sin(freqs).astype(np.float32), (4, 1))

    p2 = np.zeros((128, 128), dtype=np.float32)
    for p in list(range(0, 32)) + list(range(64, 96)):
        p2[p + 32, p] = -1.0
    for p in list(range(32, 64)) + list(range(96, 128)):
        p2[p - 32, p] = 1.0

    ident = np.eye(64, dtype=np.float32)
    ones32 = np.ones((128, (R // KB_W) * 64), dtype=np.float32)

    masks = np.zeros((128, 4, 2 * QS_W), dtype=np.float32)
    for t in range(4):
        m = (np.arange(QS_W)[None, :] >= (t * 128 + np.arange(128))[:, None])
        masks[:, t, :] = np.tile(m.astype(np.float32), (1, 2))

    # Wo full, tiled for stage 3: wo_t[p, o, d, c] = Wo[d*128+p, o*128+c]
    wo_t = np.ascontiguousarray(
        Wo.astype(np.float32).reshape(OB, 128, OB, 128).transpose(1, 2, 0, 3)
    ).astype(mybir.dt.np(mybir.dt.bfloat16))
    bo_t = np.ascontiguousarray(bo.astype(np.float32).reshape(OB, 128).T)

    in_maps = []
    for c in range(NC):
        wq_c = np.empty((D, 256), dtype=np.float32)
        for cb in range(2):
            for u in range(2):
                h = 4 * c + 2 * cb + u
                wq_c[:, cb * 128 + u * 64: cb * 128 + (u + 1) * 64] = Wq[:, h * 64 + perm]
        wq_t = np.ascontiguousarray(
            wq_c.reshape(D // 128, 128, 256).transpose(1, 0, 2))
        wkv_c = np.empty((D, 128), dtype=np.float32)
        wkv_c[:, 0:64] = Wk[:, c * 64 + perm]
        wkv_c[:, 64:128] = Wv[:, c * 64: (c + 1) * 64]
        wkv_t = np.ascontiguousarray(
            wkv_c.reshape(D // 128, 128, 128).transpose(1, 0, 2))
        bv_c = bv[c * 64:(c + 1) * 64].astype(np.float32).reshape(HD, 1)
        in_maps.append({
            "xta": xta, "xtb": xtb, "wq": wq_t, "wkv": wkv_t, "wo": wo_t,
            "bv": bv_c, "bo": bo_t, "c4h": c4h, "s4h": s4h,
            "p2": p2, "ident": ident, "masks": masks, "ones32": ones32,
        })
    return in_maps


def _run(in_maps, trace=False):
    if "nc" not in _CACHE:
        _CACHE["nc"] = _build()
    try:
        return run_bass_kernel_spmd(_CACHE["nc"], in_maps,
                                    core_ids=list(range(NC)), trace=trace)
    except Exception:
        # transient device wedge happens occasionally; one retry clears it
        return run_bass_kernel_spmd(_CACHE["nc"], in_maps,
                                    core_ids=list(range(NC)), trace=trace)


def _assemble(res):
    # core c's y_sh is yT for rows [512c, 512(c+1)) of the flattened (B*S) dim
    y = np.concatenate([res.results[c]["y_sh"].T for c in range(NC)], axis=0)
    return np.ascontiguousarray(y).reshape(B, S, D).astype(np.float32)


def kernel(x, Wq, Wk, Wv, bv, Wo, bo, mask):
    """Full inputs -> full output (B, S, D). `mask` is the causal tril mask
    from setup_inputs; causality is hardcoded so it is not shipped to device."""
    in_maps = _host_prep(np.asarray(x), np.asarray(Wq), np.asarray(Wk),
                         np.asarray(Wv), np.asarray(bv), np.asarray(Wo),
                         np.asarray(bo))
    res = _run(in_maps, trace=False)
    return _assemble(res)


def kernel_timed(x, Wq, Wk, Wv, bv, Wo, bo, mask):
    """Like kernel() but with NTFF tracing; returns (y, exec_time_ns)."""
    in_maps = _host_prep(np.asarray(x), np.asarray(Wq), np.asarray(Wk),
                         np.asarray(Wv), np.asarray(bv), np.asarray(Wo),
                         np.asarray(bo))
    res = _run(in_maps, trace=True)
    _CACHE["last_res"] = res
    return _assemble(res), res.exec_time_ns
